# revision 14
# baseline (speedup 1.0000x reference)
"""Trainium2 Bass kernel for nn_Decoder (2-layer LSTM decoder + vocab head).

Computation (matches reference.py):
  embeds = emb[sentence]                      [B, T, E]
  x = concat(features, embeds[:, :-1])        [B, T, E]
  h0 = LSTM0(x), h1 = LSTM1(h0)               [B, T, H]
  out = (h1 @ fc_W.T + fc_b).transpose(0,2,1) [B, V, T]

Sharding (8 NeuronCores, SPMD):
  - recurrence replicated (sequential, weight-load bound; per-step
    collectives are latency-infeasible at ~5us/AllGather floor);
  - fc vocab dim sharded 8 ways (4000 rows/core padded to 4096), per-core
    output [V_loc, T, B], gathered + transposed on host;
  - the input projections xp0/xp1 (the only big feed-forward matmuls that
    were replicated) are gate-sharded 8 ways: each core computes 2 of 16
    gate chunks for all tokens and the slices are AllGathered through HBM
    bounce buffers (xp0: one 6.3MB-out AG up front; xp1: one AG per 4-step
    group, pipelined LAG=6 steps ahead of rec1).  xp0 slabs 0-7 are
    computed replicated so the recurrence can start before the first AG
    lands.

Perf structure (v3): PE-bound throughout; all work is emitted as small
units interleaved between recurrence steps so the PE never idles on the
ACT/DVE gate chain:
  - W_hh in fp8-e3m4 (x128 host-side scale; g-gate rows x2 more): FWL
    ingests 4 fp8/cycle, so the FD=64 recurrent LDWEIGHTS+MATMUL pair
    runs ~30ns vs ~52ns for bf16.  The xp ring stores 32*(xp+b); the
    identity fold-in matrix is 4*I (fp8), and the gate sigmoid applies
    scale=1/128.
  - one merged ACT sigmoid per step-layer (i,f,g,o order, g rows doubled,
    tanh(x)=2*sigmoid(2x)-1 fixed by a 4x-mode DVE tensor_scalar);
  - fc is t-major: unit (v-tile, 8-step block) reads hist1 directly and
    streams out as rec1 completes each block, overlapping the output DMA
    with the LSTM.
"""

import numpy as np
import ml_dtypes

# ---------------------------------------------------------------------------
# Workaround: this walrus build caps instructions at ONE embedded sync wait
# ("Too many sync wait commands"); hoist excess waits onto same-engine NoOp
# carriers in the serialized BIR (identical semantics).
# ---------------------------------------------------------------------------
import orjson
import concourse.tile as tile

_MAXW = 1


def _split_waits_json(b: bytes) -> bytes:
    d = orjson.loads(b)
    for f in d["functions"]:
        for blk in f["blocks"]:
            out = []
            for inst in blk["instructions"]:
                si = inst.get("sync_info")
                if si:
                    w = si.get("on_wait") or []
                    if len(w) > _MAXW:
                        for i, wt in enumerate(w[:-_MAXW]):
                            out.append(
                                {
                                    "debug": inst.get("debug", 0),
                                    "engine": inst["engine"],
                                    "ins": [],
                                    "outs": [],
                                    "name": f"{inst['name']}-hw{i}",
                                    "opcode": "NoOp",
                                    "sync_info": {"on_update": [], "on_wait": [wt]},
                                }
                            )
                        si["on_wait"] = w[-_MAXW:]
                out.append(inst)
            blk["instructions"] = out
    return orjson.dumps(d)


def _patch_serialization(nc):
    orig = nc.to_json_bytes
    nc.to_json_bytes = lambda: _split_waits_json(orig())
    return nc


import concourse.bass as bass
import concourse.mybir as mybir
from concourse.bass import ts, ds
from concourse.bass_utils import run_bass_kernel_spmd

F32 = mybir.dt.float32
BF16 = mybir.dt.bfloat16
FP8E3 = mybir.dt.float8e3
AF = mybir.ActivationFunctionType
ALU = mybir.AluOpType
BF16_NP = ml_dtypes.bfloat16
FP8E3_NP = ml_dtypes.float8_e3m4

E, H, V, B, T = 512, 512, 32000, 64, 32
G = 4 * H
KC = 4
NCORES = 8
VPAD = 4096
NV = VPAD // 128
NTOK = B * T
LAG = 6                      # rec1 runs LAG steps behind rec0
WSCALE = 128.0               # fp8 weight scale
RSCALE = 32.0                # xp-ring scale
RG = [list(range(NCORES))]


def _build_nc():
    nc = bass.Bass(num_devices=NCORES)

    xT_d = nc.dram_tensor("xT", [128, KC, NTOK], BF16, kind="ExternalInput")
    wih0_d = nc.dram_tensor("wih0T", [128, KC, G], BF16, kind="ExternalInput")
    whh0_d = nc.dram_tensor("whh0T", [128, KC, G], FP8E3, kind="ExternalInput")
    whh1_d = nc.dram_tensor("whh1T", [128, KC, G], FP8E3, kind="ExternalInput")
    wih0l_d = nc.dram_tensor("wih0l", [128, KC, 256], BF16, kind="ExternalInput")
    wih1l_d = nc.dram_tensor("wih1l", [128, KC, 256], BF16, kind="ExternalInput")
    b0_d = nc.dram_tensor("b0", [128, 16], F32, kind="ExternalInput")
    b1_d = nc.dram_tensor("b1", [128, 16], F32, kind="ExternalInput")
    b0l_d = nc.dram_tensor("b0l", [128, 2], F32, kind="ExternalInput")
    b1l_d = nc.dram_tensor("b1l", [128, 2], F32, kind="ExternalInput")
    ident_d = nc.dram_tensor("ident", [128, 128], FP8E3, kind="ExternalInput")
    fcw_d = nc.dram_tensor("fcwT", [128, KC, VPAD], BF16, kind="ExternalInput")
    fcb_d = nc.dram_tensor("fcb", [128, NV], F32, kind="ExternalInput")
    out_d = nc.dram_tensor("out", [VPAD, T, B], F32, kind="ExternalOutput")

    with tile.TileContext(nc) as tc:
        with (
            tc.tile_pool(name="consts", bufs=1) as consts,
            tc.tile_pool(name="state", bufs=1) as state,
            tc.tile_pool(name="fcpool", bufs=1) as fcpool,
            tc.tile_pool(name="fcstage", bufs=3) as fcstage,
            tc.tile_pool(name="dram", bufs=1, space="DRAM") as dram,
            tc.tile_pool(name="ps_gates", bufs=2, space="PSUM") as ps_gates,
            tc.tile_pool(name="ps_big", bufs=4, space="PSUM") as ps_big,
        ):
            b0_sb = consts.tile([128, 16], F32, tag="b0")
            b1_sb = consts.tile([128, 16], F32, tag="b1")
            b0l_sb = consts.tile([128, 2], F32, tag="b0l")
            b1l_sb = consts.tile([128, 2], F32, tag="b1l")
            fcb_sb = consts.tile([128, NV], F32, tag="fcb")
            ident = consts.tile([128, 128], FP8E3, tag="ident")

            hist0 = consts.tile([128, KC, T, B], BF16, tag="hist0")
            hist1 = consts.tile([128, KC, T, B], BF16, tag="hist1")
            # g-major rings: [p, g, slot, B] so AG reloads are per-g DMAs
            xp0r = consts.tile([128, 16, 16, B], BF16, tag="xp0r")
            xp1r = consts.tile([128, 16, 8, B], BF16, tag="xp1r")
            # local-shard staging (this core's 2 gate chunks)
            xp0s = consts.tile([128, 2, 24, B], BF16, tag="xp0s")
            xp1s = consts.tile([128, 2, 4, B], BF16, tag="xp1s")

            wih0l_sb = consts.tile([128, KC, 256], BF16, tag="wih0l")
            wih1l_sb = consts.tile([128, KC, 256], BF16, tag="wih1l")

            fcw_sb = fcpool.tile([128, KC, VPAD], BF16, tag="fcw")

            # AllGather bounce buffers (HBM)
            agi0 = dram.tile([2, 128, 24, B], BF16, tag="agi0")
            ago0 = dram.tile([16, 128, 24, B], BF16, tag="ago0",
                             addr_space="Shared")
            agi1 = [dram.tile([2, 128, 4, B], BF16, tag=f"agi1_{k}", name=f"agi1_{k}")
                    for k in range(8)]
            ago1 = [dram.tile([16, 128, 4, B], BF16, tag=f"ago1_{k}",
                              name=f"ago1_{k}", addr_space="Shared")
                    for k in range(8)]

            st = []
            for l in range(2):
                cT = state.tile([128, KC, B], F32, tag=f"cT{l}", name=f"cT{l}")
                gates = state.tile([128, 16, B], BF16, tag=f"gates{l}")
                g2 = state.tile([128, KC, B], BF16, tag=f"g2{l}")
                tmp1 = state.tile([128, KC, B], F32, tag=f"tmp1{l}")
                tmp2 = state.tile([128, KC, B], F32, tag=f"tmp2{l}")
                tanh_c = state.tile([128, KC, B], F32, tag=f"tanhc{l}")
                st.append(dict(cT=cT, gates=gates, g2=g2, tmp1=tmp1, tmp2=tmp2,
                               tanh_c=tanh_c))

            def xp_epilogue(dst, ps, nslab, bias_ap, use_act):
                if use_act:
                    nc.scalar.activation(
                        out=dst, in_=ps[:, 0:nslab, :], func=AF.Identity,
                        bias=bias_ap, scale=RSCALE,
                    )
                else:
                    nc.vector.tensor_scalar(
                        out=dst, in0=ps[:, 0:nslab, :],
                        scalar1=RSCALE, scalar2=bias_ap,
                        op0=ALU.mult, op1=ALU.add,
                    )

            def xp_unit(w_sb, rhs_slice, bias_sb, ring, s0, nslab, g):
                """Replicated-head unit -> ring[:, g, slot, :]."""
                ps = ps_big.tile([128, 8, B], F32, tag="ps512")
                for kc in range(KC):
                    nc.tensor.matmul(
                        ps[:, 0:nslab, :],
                        w_sb[:, kc, ts(g, 128)],
                        rhs_slice(kc, s0 * B, nslab * B),
                        start=(kc == 0),
                        stop=(kc == KC - 1),
                    )
                dst = ring[:, g, ds(s0 % ring.shape[2], nslab), :]
                xp_epilogue(dst, ps, nslab, bias_sb[:, g : g + 1], g % 2 == 0)

            def xp_shard_unit(wl_sb, rhs_slice, bl_sb, stage, s0, soff, gl,
                              nslab):
                """Local gate-chunk gl of slabs [s0,s0+nslab) -> staging."""
                ps = ps_big.tile([128, 8, B], F32, tag="ps512")
                for kc in range(KC):
                    nc.tensor.matmul(
                        ps[:, 0:nslab, :],
                        wl_sb[:, kc, ts(gl, 128)],
                        rhs_slice(kc, s0 * B, nslab * B),
                        start=(kc == 0),
                        stop=(kc == KC - 1),
                    )
                dst = stage[:, gl, ds(soff, nslab), :]
                xp_epilogue(dst, ps, nslab, bl_sb[:, gl : gl + 1], gl % 2 == 0)

            def fc_unit(u):
                tb, v = u // NV, u % NV
                ps = ps_big.tile([128, 8, B], F32, tag="ps512")
                for kc in range(KC):
                    nc.tensor.matmul(
                        ps,
                        fcw_sb[:, kc, ts(v, 128)],
                        hist1[:, kc, ts(tb, 8), :],
                        start=(kc == 0),
                        stop=(kc == KC - 1),
                    )
                ot = fcstage.tile([128, 8, B], F32, tag="ot")
                if u % 2 == 0:
                    nc.scalar.activation(
                        out=ot, in_=ps, func=AF.Identity,
                        bias=fcb_sb[:, v : v + 1], scale=1.0,
                    )
                else:
                    nc.vector.tensor_scalar_add(ot, ps, fcb_sb[:, v : v + 1])
                eng = (nc.sync, nc.scalar)[u % 2]
                eng.dma_start(out=out_d[ts(v, 128), ts(tb, 8), :], in_=ot)

            def rec_step(l, t, whh_sb, ring, hist):
                s = st[l]
                xsl = ring[:, :, t % ring.shape[2], :]
                ps = ps_gates.tile([128, 16, B], F32, tag="psg")
                for half in (0, 1):
                    if t > 0:
                        for j in range(8):
                            gc = half * 8 + j
                            for kc in range(KC):
                                nc.tensor.matmul(
                                    ps[:, gc, :],
                                    whh_sb[:, kc, ts(gc, 128)],
                                    hist[:, kc, t - 1, :],
                                    start=(j == 0 and kc == 0),
                                    stop=False,
                                    skip_group_check=True,
                                )
                    nc.tensor.matmul(
                        ps[:, ts(half, 8), :],
                        ident,
                        xsl[:, ts(half, 8), :],
                        start=(t == 0),
                        stop=True,
                        skip_group_check=True,
                    )
                g = s["gates"]
                nc.scalar.activation(g, ps, func=AF.Sigmoid, scale=1.0 / WSCALE)
                nc.vector.tensor_scalar(
                    out=s["g2"], in0=g[:, 8:12, :],
                    scalar1=2.0, scalar2=1.0, op0=ALU.mult, op1=ALU.subtract,
                )
                if t == 0:
                    nc.vector.tensor_mul(s["cT"], g[:, 0:4, :], s["g2"])
                else:
                    nc.vector.tensor_mul(s["tmp1"], g[:, 0:4, :], s["g2"])
                    nc.vector.tensor_mul(s["tmp2"], g[:, 4:8, :], s["cT"])
                    nc.vector.tensor_add(s["cT"], s["tmp1"], s["tmp2"])
                nc.scalar.activation(s["tanh_c"], s["cT"], func=AF.Tanh)
                nc.vector.tensor_mul(hist[:, :, t, :], g[:, 12:16, :], s["tanh_c"])

            fc_state = {"done": 0, "ready": 0}

            def fc_ready(s_done):
                fc_state["ready"] = NV * ((s_done + 1) // 8)

            def fc_emit(k):
                n = min(fc_state["done"] + k, fc_state["ready"])
                for u in range(fc_state["done"], n):
                    fc_unit(u)
                fc_state["done"] = n

            def ring_reload(ring, ago, g, slot0, nslab, src_off, eng):
                eng.dma_start(
                    out=ring[:, g, ds(slot0, nslab), :],
                    in_=ago[g, :, ds(src_off, nslab), :],
                )

            with tc.tile_pool(name="wpool", bufs=1) as wpool:
                whh0_sb = wpool.tile([128, KC, G], FP8E3, tag="whh0")
                whh1_sb = wpool.tile([128, KC, G], FP8E3, tag="whh1")

                with tc.tile_pool(name="inpool", bufs=1) as inpool:
                    xT_sb = inpool.tile([128, KC, NTOK], BF16, tag="xT")
                    wih0_sb = inpool.tile([128, KC, G], BF16, tag="wih0")
                    # smallest first-needed pieces lead each queue
                    nc.scalar.dma_start(out=b0l_sb, in_=b0l_d[:])
                    nc.scalar.dma_start(out=b0_sb, in_=b0_d[:])
                    nc.scalar.dma_start(out=wih0l_sb, in_=wih0l_d[:])
                    nc.sync.dma_start(out=xT_sb[:, :, 0:512],
                                      in_=xT_d[:, :, 0:512])
                    nc.gpsimd.dma_start(out=whh0_sb, in_=whh0_d[:])
                    nc.scalar.dma_start(out=wih0_sb, in_=wih0_d[:])
                    nc.sync.dma_start(out=xT_sb[:, :, 512:NTOK],
                                      in_=xT_d[:, :, 512:NTOK])
                    nc.scalar.dma_start(out=wih1l_sb, in_=wih1l_d[:])
                    nc.scalar.dma_start(out=b1l_sb, in_=b1l_d[:])
                    nc.scalar.dma_start(out=b1_sb, in_=b1_d[:])
                    nc.scalar.dma_start(out=ident, in_=ident_d[:])
                    nc.scalar.dma_start(out=fcb_sb, in_=fcb_d[:])

                    xp0_rhs = lambda kc, n0, nt: xT_sb[:, kc, ds(n0, nt)]
                    xp1_rhs = lambda kc, n0, nt: hist0[:, kc, ds(n0 // B, nt // B), :]

                    # ---- xp0 shard (slabs 8-31, this core's 2 chunks) ----
                    for sg in range(6):
                        for gl in range(2):
                            xp_shard_unit(wih0l_sb, xp0_rhs, b0l_sb, xp0s,
                                          8 + 4 * sg, 4 * sg, gl, 4)
                    for gl in range(2):
                        nc.sync.dma_start(out=agi0[gl], in_=xp0s[:, gl, :, :])
                    nc.gpsimd.collective_compute(
                        "AllGather", ALU.bypass, replica_groups=RG,
                        ins=[agi0.opt()], outs=[ago0.opt()],
                    )
                    nc.gpsimd.dma_start(out=whh1_sb, in_=whh1_d[:])
                    nc.gpsimd.dma_start(out=fcw_sb, in_=fcw_d[:])

                    # ---- replicated head: xp0 slabs 0-3 (then 4-7 at t=0/1)
                    for g in range(16):
                        xp_unit(wih0_sb, xp0_rhs, b0_sb, xp0r, 0, 4, g)
                    # reload slabs 8-15 -> slots 8-15 once the AG lands
                    for g in range(16):
                        ring_reload(xp0r, ago0, g, 8, 8, 0,
                                    (nc.sync, nc.scalar)[g % 2])

                    for t in range(24):
                        rec_step(0, t, whh0_sb, xp0r, hist0)
                        if t < 2:
                            for g in range(8 * t, 8 * t + 8):
                                xp_unit(wih0_sb, xp0_rhs, b0_sb, xp0r, 4, 4, g)
                        if t == 7:
                            for g in range(16):
                                ring_reload(xp0r, ago0, g, 0, 8, 8,
                                            (nc.sync, nc.scalar)[g % 2])
                        if t == 15:
                            for g in range(16):
                                ring_reload(xp0r, ago0, g, 8, 8, 16,
                                            (nc.sync, nc.scalar)[g % 2])
                        if t % 4 == 3:
                            k = t // 4
                            for gl in range(2):
                                xp_shard_unit(wih1l_sb, xp1_rhs, b1l_sb, xp1s,
                                              4 * k, 0, gl, 4)
                            for gl in range(2):
                                nc.sync.dma_start(out=agi1[k][gl],
                                                  in_=xp1s[:, gl, :, :])
                            nc.gpsimd.collective_compute(
                                "AllGather", ALU.bypass, replica_groups=RG,
                                ins=[agi1[k].opt()], outs=[ago1[k].opt()],
                            )
                        elif t % 4 == 0 and t > 0:
                            k = t // 4 - 1
                            for g in range(16):
                                ring_reload(xp1r, ago1[k], g, (4 * k) % 8, 4,
                                            0, (nc.sync, nc.scalar)[g % 2])
                        if t >= LAG:
                            rec_step(1, t - LAG, whh1_sb, xp1r, hist1)
                            fc_ready(t - LAG)
                            fc_emit(4)
                for t in range(24, T):
                    rec_step(0, t, whh0_sb, xp0r, hist0)
                    if t % 4 == 3:
                        k = t // 4
                        for gl in range(2):
                            xp_shard_unit(wih1l_sb, xp1_rhs, b1l_sb, xp1s,
                                          4 * k, 0, gl, 4)
                        for gl in range(2):
                            nc.sync.dma_start(out=agi1[k][gl],
                                              in_=xp1s[:, gl, :, :])
                        nc.gpsimd.collective_compute(
                            "AllGather", ALU.bypass, replica_groups=RG,
                            ins=[agi1[k].opt()], outs=[ago1[k].opt()],
                        )
                    elif t % 4 == 0:
                        k = t // 4 - 1
                        for g in range(16):
                            ring_reload(xp1r, ago1[k], g, (4 * k) % 8, 4, 0,
                                        (nc.sync, nc.scalar)[g % 2])
                    rec_step(1, t - LAG, whh1_sb, xp1r, hist1)
                    fc_ready(t - LAG)
                    fc_emit(4)
                for g in range(16):
                    ring_reload(xp1r, ago1[7], g, 4, 4, 0,
                                (nc.sync, nc.scalar)[g % 2])
                for s_ in range(T - LAG, T):
                    rec_step(1, s_, whh1_sb, xp1r, hist1)
                    fc_ready(s_)
                    fc_emit(5)
            fc_emit(4 * NV)
    return _patch_serialization(nc)


def _to_k128(W, dtype):
    """W [out_dim, K] -> [128, K//128, out_dim] with result[p,kc,g]=W[g,kc*128+p]."""
    K = W.shape[1]
    return np.ascontiguousarray(
        W.T.reshape(K // 128, 128, -1).transpose(1, 0, 2)
    ).astype(dtype)


_NC_CACHE = None
RUN_KWARGS = {}
LAST_RESULT = None


def kernel(
    sentence,
    features,
    lengths,
    emb,
    W_ih0,
    W_hh0,
    b_ih0,
    b_hh0,
    W_ih1,
    W_hh1,
    b_ih1,
    b_hh1,
    fc_W,
    fc_b,
):
    global _NC_CACHE, LAST_RESULT
    sentence = np.asarray(sentence).astype(np.int64)
    features = np.asarray(features, dtype=np.float32)
    emb = np.asarray(emb, dtype=np.float32)

    embeds = emb[sentence[:, : T - 1]]
    x = np.concatenate([features[:, None, :], embeds], axis=1)
    xT = np.ascontiguousarray(x.transpose(2, 1, 0).reshape(E, NTOK))
    xT_p = np.ascontiguousarray(
        xT.reshape(KC, 128, NTOK).transpose(1, 0, 2)
    ).astype(BF16_NP)

    def prep_layer(W_ih, W_hh, b_ih, b_hh):
        wih = np.asarray(W_ih, np.float32).copy()
        whh = np.asarray(W_hh, np.float32).copy()
        b = (np.asarray(b_ih, np.float32) + np.asarray(b_hh, np.float32)).copy()
        wih[2 * H : 3 * H] *= 2.0
        whh[2 * H : 3 * H] *= 2.0
        b[2 * H : 3 * H] *= 2.0
        wih_p = _to_k128(wih, BF16_NP)
        whh_p = _to_k128(whh * WSCALE, FP8E3_NP)
        b_p = np.ascontiguousarray((b * RSCALE).reshape(16, 128).T)
        return wih_p, whh_p, b_p

    wih0, whh0, b0 = prep_layer(W_ih0, W_hh0, b_ih0, b_hh0)
    wih1, whh1, b1 = prep_layer(W_ih1, W_hh1, b_ih1, b_hh1)

    fc_W = np.asarray(fc_W, np.float32)
    fc_b = np.asarray(fc_b, np.float32)
    vloc = V // NCORES

    common = {
        "xT": xT_p,
        "wih0T": wih0,
        "whh0T": whh0,
        "whh1T": whh1,
        "b0": b0,
        "b1": b1,
        "ident": (np.eye(128, dtype=np.float32) * (WSCALE / RSCALE)).astype(
            FP8E3_NP
        ),
    }
    in_maps = []
    for c in range(NCORES):
        wslice = np.zeros((VPAD, E), np.float32)
        wslice[:vloc] = fc_W[c * vloc : (c + 1) * vloc]
        bslice = np.zeros(VPAD, np.float32)
        bslice[:vloc] = fc_b[c * vloc : (c + 1) * vloc]
        wc = _to_k128(wslice, BF16_NP)
        bc = np.ascontiguousarray(bslice.reshape(NV, 128).T)
        gsl = slice(2 * c * 128, (2 * c + 2) * 128)
        in_maps.append({
            **common,
            "fcwT": wc,
            "fcb": bc,
            "wih0l": np.ascontiguousarray(wih0[:, :, gsl]),
            "wih1l": np.ascontiguousarray(wih1[:, :, gsl]),
            "b0l": np.ascontiguousarray(b0[:, 2 * c : 2 * c + 2]),
            "b1l": np.ascontiguousarray(b1[:, 2 * c : 2 * c + 2]),
        })

    if _NC_CACHE is None:
        _NC_CACHE = _build_nc()

    res = run_bass_kernel_spmd(
        _NC_CACHE, in_maps, core_ids=list(range(NCORES)), **RUN_KWARGS
    )
    LAST_RESULT = res
    full = np.concatenate(
        [res.results[c]["out"][:vloc] for c in range(NCORES)], axis=0
    )  # [V, T, B]
    return np.ascontiguousarray(full.transpose(2, 0, 1))
